# revision 1
# baseline (speedup 1.0000x reference)
# 2D DCT-II [4096,4096] fp32 on 8 NeuronCores — v2 "folded dense".
#
# DCT even/odd fold: C[i, M-1-r] = (-1)^i C[i, r]  =>
#   U[2i'']   = sum_{r<2048} C[2i'', r]   * (X[r] + X[4095-r])
#   U[2i''+1] = sum_{r<2048} C[2i''+1, r] * (X[r] - X[4095-r])
# halving matmul work per pass. Orientation: data tiles are lhsT (stationary),
# cos-weights are rhs (moving), so pass-1 emits U^T tiles [c-part, i-free] and
# pass-2 consumes them directly after the AllToAll with no on-chip transposes.
# Intermediate rows travel in "folded" order [even-2048 || odd-2048]; the final
# store un-permutes via strided row/col addressing.
import numpy as np
from einops import rearrange
import concourse.bacc as bacc
import concourse.tile as tile
import concourse.mybir as mybir
from concourse import bass_utils

M = N = 4096
NC = 8
CB = 512          # columns per core (pass 1) / rows per core (pass 2)
KH = M // 2       # 2048 folded contraction length
KT = KH // 128    # 16 K-tiles
NCH = KH // 512   # 4 N-chunks of 512 per block

_BUILT = {}


def build_nc(repeat=1):
    dt = mybir.dt
    f32r = dt.float32r
    nc = bacc.Bacc("TRN2", target_bir_lowering=False, debug=False, num_devices=NC)

    xf = nc.dram_tensor("xf", [128, KT, CB], f32r, kind="ExternalInput")  # X[r<2048, cols]
    xr = nc.dram_tensor("xr", [128, KT, CB], f32r, kind="ExternalInput")  # X[4095-r, cols]
    we = nc.dram_tensor("we", [128, KT, KH], f32r, kind="ExternalInput")  # C[2k, r'].T
    jrev = nc.dram_tensor("jrev", [128, 128], f32r, kind="ExternalInput")  # anti-identity
    wo = nc.dram_tensor("wo", [128, KT, KH], f32r, kind="ExternalInput")  # C[2k+1, r'].T
    y = nc.dram_tensor("y", [CB, N], f32r, kind="ExternalOutput")         # Y[rows_mine] true order

    with tile.TileContext(nc) as tc:
        with tc.tile_pool(name="dram", bufs=1, space="DRAM") as dram:
            z1 = dram.tile([NC, CB, CB], f32r)  # [chunk, c-local, folded-row]
            z2 = dram.tile([NC, CB, CB], f32r)
            for _rep in range(repeat):
                # ================= pass 1 =================
                with (
                    tc.tile_pool(name="xstage", bufs=3) as xstage,
                    tc.tile_pool(name="xfold", bufs=1) as xfold,
                    tc.tile_pool(name="wsl", bufs=2) as wsl,
                    tc.tile_pool(name="ps1", bufs=4, space="PSUM") as ps1,
                    tc.tile_pool(name="ev1", bufs=3) as ev1,
                ):
                    xp = xfold.tile([128, KT, CB], f32r, tag="xp")
                    xm = xfold.tile([128, KT, CB], f32r, tag="xm")
                    for k in range(KT):
                        tf = xstage.tile([128, CB], f32r, tag="tf")
                        tr = xstage.tile([128, CB], f32r, tag="tr")
                        nc.sync.dma_start(out=tf[:], in_=xf[:, k])
                        nc.sync.dma_start(out=tr[:], in_=xr[:, k])
                        nc.vector.tensor_add(xp[:, k], tf[:], tr[:])
                        nc.vector.tensor_sub(xm[:, k], tf[:], tr[:])
                    for blk, (xb, wb) in enumerate(((xp, we), (xm, wo))):
                        for nch in range(NCH):
                            wt = wsl.tile([128, KT, 512], f32r, tag="wslab")
                            nc.sync.dma_start(out=wt[:], in_=wb[:, :, nch * 512:(nch + 1) * 512])
                            for cm in range(CB // 128):
                                psum = ps1.tile([128, 512], dt.float32, tag="ps")
                                for k in range(KT):
                                    nc.tensor.matmul(psum[:], xb[:, k, cm * 128:(cm + 1) * 128],
                                                     wt[:, k],
                                                     start=(k == 0), stop=(k == KT - 1))
                                ev = ev1.tile([128, 512], f32r, tag="ev")
                                nc.vector.tensor_copy(ev[:], psum[:])
                                for piece in range(2):
                                    ch = nch * 2 + piece
                                    fold0 = blk * 256
                                    nc.sync.dma_start(
                                        out=z1[ch, cm * 128:(cm + 1) * 128,
                                               fold0:fold0 + 256],
                                        in_=ev[:, piece * 256:(piece + 1) * 256])

                # ================= A2A =================
                nc.gpsimd.collective_compute(
                    "AllToAll", mybir.AluOpType.bypass,
                    replica_groups=[list(range(NC))],
                    ins=[z1[:].opt()], outs=[z2[:].opt()])

                # ================= pass 2 =================
                with (
                    tc.tile_pool(name="zstage", bufs=3) as zstage,
                    tc.tile_pool(name="zfold", bufs=1) as zfold,
                    tc.tile_pool(name="wsl2", bufs=2) as wsl2,
                    tc.tile_pool(name="ps2", bufs=4, space="PSUM") as ps2,
                    tc.tile_pool(name="yt", bufs=1) as ytp,
                ):
                    zp = zfold.tile([128, KT, CB], f32r, tag="zp")
                    zm = zfold.tile([128, KT, CB], f32r, tag="zm")
                    jt = zstage.tile([128, 128], f32r, tag="jrev")
                    nc.sync.dma_start(out=jt[:], in_=jrev[:])
                    z2f = z2[:].rearrange("s c r -> (s c) r")
                    for kt in range(KT):
                        tf = zstage.tile([128, CB], f32r, tag="tf2")
                        ta = zstage.tile([128, CB], f32r, tag="ta2")
                        # forward cols: c = 128*kt + p
                        nc.sync.dma_start(out=tf[:], in_=z2f[kt * 128:(kt + 1) * 128, :])
                        # mirror block, ascending: c = 3968 - 128*kt + p
                        nc.sync.dma_start(out=ta[:],
                                          in_=z2f[3968 - 128 * kt:4096 - 128 * kt, :])
                        # reverse partitions: tr = J @ ta  (J = anti-identity)
                        prr = ps2.tile([128, CB], dt.float32, tag="prr")
                        nc.tensor.matmul(prr[:], jt[:], ta[:], start=True, stop=True)
                        tr = zstage.tile([128, CB], f32r, tag="tr2")
                        nc.vector.tensor_copy(tr[:], prr[:])
                        nc.vector.tensor_add(zp[:, kt], tf[:], tr[:])
                        nc.vector.tensor_sub(zm[:, kt], tf[:], tr[:])
                    for half in range(2):
                        ythalf = []
                        for rm in range(CB // 128):
                            ytl = ytp.tile([128, N // 2], f32r, tag=f"yt{rm}")
                            ythalf.append(ytl)
                        for blk, (zb, wb) in enumerate(((zp, we), (zm, wo))):
                            for nch2 in range(2):
                                nch = half * 2 + nch2
                                wt = wsl2.tile([128, KT, 512], f32r, tag="wslab2")
                                nc.sync.dma_start(
                                    out=wt[:],
                                    in_=wb[:, :, nch * 512:(nch + 1) * 512])
                                for rm in range(CB // 128):
                                    psum = ps2.tile([128, 512], dt.float32, tag="ps2")
                                    for k in range(KT):
                                        nc.tensor.matmul(
                                            psum[:], zb[:, k, rm * 128:(rm + 1) * 128],
                                            wt[:, k],
                                            start=(k == 0), stop=(k == KT - 1))
                                    # k-true-local = 2*(nch2*512 + k'') + blk
                                    dst = ythalf[rm][:].rearrange("p (a b) -> p a b", b=2)
                                    nc.vector.tensor_copy(
                                        dst[:, nch2 * 512:(nch2 + 1) * 512, blk], psum[:])
                        # store this column-half; fp->true rows:
                        # rm 0/1: true = 2*fp ; rm 2/3: true = 2*(fp-256)+1
                        yv = y[:].rearrange("(a b) n -> a b n", b=2)  # [2048, 2, 4096]
                        for rm in range(CB // 128):
                            if rm < 2:
                                out_ap = yv[rm * 128:(rm + 1) * 128, 0,
                                            half * 2048:(half + 1) * 2048]
                            else:
                                out_ap = yv[(rm - 2) * 128:(rm - 1) * 128, 1,
                                            half * 2048:(half + 1) * 2048]
                            nc.sync.dma_start(out=out_ap, in_=ythalf[rm][:])

    nc.compile()
    return nc


def _weights():
    n = np.arange(M, dtype=np.float64)
    k = np.arange(M, dtype=np.float64)
    C = np.cos(np.pi * (2.0 * n[None, :] + 1.0) * k[:, None] / (2.0 * M))
    We = np.ascontiguousarray(C[0::2, :KH].T).astype(np.float32)  # [r', i'']
    Wo = np.ascontiguousarray(C[1::2, :KH].T).astype(np.float32)
    return We, Wo


def tile3(a):
    return np.ascontiguousarray(rearrange(a, "(m p) n -> p m n", p=128))


def kernel(x, expkM=None, expkN=None, trace=False):
    x = np.asarray(x, dtype=np.float32)
    if "nc" not in _BUILT:
        _BUILT["nc"] = build_nc()
        We, Wo = _weights()
        _BUILT["we"] = tile3(We)
        _BUILT["wo"] = tile3(Wo)
        _BUILT["jrev"] = np.ascontiguousarray(np.eye(128, dtype=np.float32)[::-1])
    nc = _BUILT["nc"]
    xrev = x[::-1, :]
    in_maps = []
    for c in range(NC):
        sl = slice(c * CB, (c + 1) * CB)
        in_maps.append({
            "xf": tile3(x[:KH, sl]),
            "xr": tile3(xrev[:KH, sl]),
            "we": _BUILT["we"],
            "wo": _BUILT["wo"],
            "jrev": _BUILT["jrev"],
        })
    res = bass_utils.run_bass_kernel_spmd(nc, in_maps, core_ids=list(range(NC)),
                                          trace=trace)
    _BUILT["last_res"] = res
    out = np.concatenate([res.results[c]["y"] for c in range(NC)], axis=0)
    return out.astype(np.float32)



# revision 23
# speedup vs baseline: 1.6201x; 1.6201x over previous
# 2D DCT-II [4096,4096] fp32 on 8 NeuronCores — v4 "bf16 resident-weight,
# fully overlapped".
#
# Same folded-dense algorithm as v2 (Makhoul even/odd fold halves each pass's
# matmul work; pencil decomposition with an AllToAll between passes), plus:
#   * everything on the wire/SBUF is bf16 (tolerance is 2e-2; PSUM accumulates
#     fp32 so error stays ~0.5%): weights DMA halves, A2A traffic halves.
#   * cos-weight matrices (16.8 MB bf16) are loaded ONCE and stay resident in
#     SBUF across both passes (v2 streamed 67 MB of fp32 weights per core).
#   * the AllToAll is split into even/odd halves (z1a/z1b): the even half
#     flies while the odd half of pass 1 is still on the PE, and pass-2's
#     even half computes while the odd A2A flies — no exposed collective.
#   * pass-2 prep (z2 staging loads, J-reversal matmuls, folds) is
#     interleaved into the preceding matmul block in kt-pairs so the PE
#     never idles at the pass/parity boundaries.
#   * queue placement is deliberate: input/weight streaming on SP (strict
#     first-use order), staging loads + tr2 + drain y stores on ACT, z1
#     stores + collectives + overlapped y stores on the Pool/SWDGE queue
#     (separate semaphore pool), folds/evac copies on DVE. A deep ev pool
#     (12 bufs) absorbs z1-store latency behind the weight stream.
#
# DCT even/odd fold: C[i, M-1-r] = (-1)^i C[i, r]  =>
#   U[2i'']   = sum_{r<2048} C[2i'', r]   * (X[r] + X[4095-r])
#   U[2i''+1] = sum_{r<2048} C[2i''+1, r] * (X[r] - X[4095-r])
# Orientation: data tiles are lhsT (stationary), cos-weights are rhs (moving),
# so pass-1 emits U^T tiles [c-part, i-free] and pass-2 consumes them directly
# after the AllToAll. Intermediate rows travel in "folded" order; the final
# store un-permutes via strided row/col addressing.
import numpy as np
import ml_dtypes
from einops import rearrange
import concourse.bacc as bacc
import concourse.tile as tile
import concourse.mybir as mybir
from concourse import bass_utils

M = N = 4096
NC = 8
CB = 512          # columns per core (pass 1) / rows per core (pass 2)
KH = M // 2       # 2048 folded contraction length
KT = KH // 128    # 16 K-tiles
NCH = KH // 512   # 4 N-chunks of 512 per block

_BUILT = {}


def build_nc(repeat=1, local_sim=False):
    dt = mybir.dt
    bf = dt.bfloat16
    nc = bacc.Bacc("TRN2", target_bir_lowering=False, debug=False, num_devices=NC)

    # packed pass-1 input: xfr[:, k, 0] = X[r<2048, cols], [:, k, 1] = X[4095-r, cols]
    xfr = nc.dram_tensor("xfr", [128, KT, 2, CB], bf, kind="ExternalInput")
    we = nc.dram_tensor("we", [128, NCH, KT, 512], bf, kind="ExternalInput")  # C[2k,:].T slabs
    wo = nc.dram_tensor("wo", [128, NCH, KT, 512], bf, kind="ExternalInput")  # C[2k+1,:].T slabs
    jrev = nc.dram_tensor("jrev", [128, 128], bf, kind="ExternalInput")  # anti-identity
    y = nc.dram_tensor("y", [CB, N], bf, kind="ExternalOutput")          # Y[rows_mine] true order

    with tile.TileContext(nc) as tc:
        with (
            tc.tile_pool(name="dram", bufs=1, space="DRAM") as dram,
            tc.tile_pool(name="wpool", bufs=1) as wpool,
            tc.tile_pool(name="foldp", bufs=1) as foldp,
            tc.tile_pool(name="xst", bufs=2) as xst,
            tc.tile_pool(name="zst", bufs=4) as zst,
            tc.tile_pool(name="evp", bufs=12) as evp,
            tc.tile_pool(name="jp", bufs=1) as jp,
            tc.tile_pool(name="ytp", bufs=1) as ytp,
            tc.tile_pool(name="psp", bufs=4, space="PSUM") as psp,
            tc.tile_pool(name="psj", bufs=3, space="PSUM") as psj,
        ):
            # z1x[ch, c_local, j]: shard for dest core ch; z1a holds even
            # output rows 2*(256*ch + j), z1b the odd rows 2*(256*ch + j)+1.
            z1a = dram.tile([NC, CB, 256], bf)
            z1b = dram.tile([NC, CB, 256], bf)
            z2a = dram.tile([NC, CB, 256], bf)
            z2b = dram.tile([NC, CB, 256], bf)

            for _rep in range(repeat):
                jt = jp.tile([128, 128], bf, tag="jt")
                nc.scalar.dma_start(out=jt[:], in_=jrev[:])
                # ---- startup: x chunks and wes0 sub-slabs interleaved so the
                # serial HBM pipe delivers (x + first weight slab) ASAP; the
                # remaining resident slabs trickle behind, ordered by first
                # use. SP and ACT queues round-robin into the DMA engines.
                wes, wos = [None] * NCH, [None] * NCH
                for nch in range(NCH):
                    wes[nch] = wpool.tile([128, KT, 512], bf, tag=f"we{nch}",
                                          name="wes")
                    wos[nch] = wpool.tile([128, KT, 512], bf, tag=f"wo{nch}",
                                          name="wos")
                xcs = []
                for c2 in range(KT // 2):
                    xc = xst.tile([128, 2, 2, CB], bf, tag="xc", name="xc")
                    nc.sync.dma_start(out=xc[:], in_=xfr[:, 2 * c2:2 * c2 + 2])
                    xcs.append(xc)
                for nch in range(NCH):
                    nc.sync.dma_start(out=wes[nch][:], in_=we[:, nch])
                for nch in range(NCH):
                    nc.sync.dma_start(out=wos[nch][:], in_=wo[:, nch])
                # ---- fold: xp = X[r]+X[4095-r], xm = X[r]-X[4095-r].
                # All adds first: blk0 needs only xp, so its matmuls are not
                # gated on the xm chain.
                xp = foldp.tile([128, KT, CB], bf, tag="fA", name="xp")
                xm = foldp.tile([128, KT, CB], bf, tag="fB", name="xm")
                for k in range(KT):
                    nc.vector.tensor_add(xp[:, k], xcs[k // 2][:, k % 2, 0],
                                         xcs[k // 2][:, k % 2, 1])
                    nc.vector.tensor_sub(xm[:, k], xcs[k // 2][:, k % 2, 0],
                                         xcs[k // 2][:, k % 2, 1])

                # ---- pass-2 prep emitters, interleaved into the preceding
                # matmul block in kt-PAIRS. Loads (on ACT) start a few groups
                # in — as soon as the half-A2A could have landed; the
                # J-reversal matmul + fold chain trails on PE/ACT/DVE so a
                # late A2A can't stall the PE FIFO mid-block.
                def emit_load(z2x, stage, kt):
                    # flat rows (s c) are c_g; zr[p, t] = row 128t + p
                    zr = z2x[:].rearrange("s (ch p) j -> p (s ch) j", p=128)
                    tf2 = zst.tile([128, 2, 256], bf, tag="tf2", name="tf2",
                                   bufs=8)
                    ta2 = zst.tile([128, 2, 256], bf, tag="ta2", name="ta2",
                                   bufs=8)
                    # forward: c_g = 128*(kt+t) + p
                    nc.scalar.dma_start(out=tf2[:], in_=zr[:, kt:kt + 2])
                    # mirror blocks (ascending t): [:,0]=mirror(kt+1), [:,1]=mirror(kt)
                    nc.scalar.dma_start(out=ta2[:], in_=zr[:, 30 - kt:32 - kt])
                    stage[kt] = (tf2, ta2)

                def emit_jfold(fz, stage, kt):
                    tf2, ta2 = stage[kt]
                    # reverse partitions: tr2 = J @ ta2  (J = anti-identity);
                    # tr2[:, t] then holds y1[4095 - r2] for kt+1-t
                    prr = psj.tile([128, 512], dt.float32, tag="pj", name="prr")
                    nc.tensor.matmul(prr[:], jt[:],
                                     ta2[:].rearrange("p t j -> p (t j)"),
                                     start=True, stop=True)
                    tr2 = zst.tile([128, 2, 256], bf, tag="tr2", name="tr2")
                    nc.scalar.copy(tr2[:].rearrange("p t j -> p (t j)"), prr[:])
                    for t in range(2):
                        nc.vector.tensor_add(fz[:, kt + t, 0:256],
                                             tf2[:, t], tr2[:, 1 - t])
                        nc.vector.tensor_sub(fz[:, kt + t, 256:512],
                                             tf2[:, t], tr2[:, 1 - t])

                def hook(z2x, fz, stage):
                    def run(g):
                        if 4 <= g < 8:
                            emit_load(z2x, stage, 4 * (g - 4))
                            emit_load(z2x, stage, 4 * (g - 4) + 2)
                        if 9 <= g < 13:
                            emit_jfold(fz, stage, 4 * (g - 9))
                            emit_jfold(fz, stage, 4 * (g - 9) + 2)
                    return run

                # ================= pass 1 =================
                # z1 stores and the collectives all ride the otherwise-idle
                # Pool/SWDGE queue, so each half-A2A waits on one clean
                # monotone counter that nothing else pollutes.
                fza = fzb = None
                stage0, stage1 = {}, {}
                for blk, (xb, wsl, z1x, z2x, ev_eng) in enumerate(
                        ((xp, wes, z1a, z2a, nc.gpsimd),
                         (xm, wos, z1b, z2b, nc.gpsimd))):
                    if blk == 1:
                        # pass-2 even-half fold tiles; prep interleaves below
                        fza = foldp.tile([128, KT, CB], bf, tag="fA", name="fza")
                        prep = hook(z2a, fza, stage0)
                    else:
                        prep = None
                    g = 0
                    for nch in range(NCH):
                        for cm in range(CB // 128):
                            psum = psp.tile([128, 512], dt.float32, tag="ps",
                                            name="ps1")
                            for k in range(KT):
                                nc.tensor.matmul(psum[:],
                                                 xb[:, k, cm * 128:(cm + 1) * 128],
                                                 wsl[nch][:, k],
                                                 start=(k == 0), stop=(k == KT - 1))
                            ev = evp.tile([128, 512], bf, tag="ev", name="ev")
                            nc.vector.tensor_copy(ev[:], psum[:])
                            for piece in range(2):
                                ev_eng.dma_start(
                                    out=z1x[2 * nch + piece,
                                            cm * 128:(cm + 1) * 128, :],
                                    in_=ev[:, piece * 256:(piece + 1) * 256])
                            if prep is not None:
                                prep(g)
                            g += 1
                    # half-A2A fires as soon as this parity's stores land;
                    # the even one overlaps the odd pass-1 matmuls.
                    if local_sim:
                        nc.gpsimd.dma_start(out=z2x[:], in_=z1x[:])
                    else:
                        nc.gpsimd.collective_compute(
                            "AllToAll", mybir.AluOpType.bypass,
                            replica_groups=[list(range(NC))],
                            ins=[z1x[:].opt()], outs=[z2x[:].opt()])

                # ================= pass 2 =================
                # core c owns 512 true rows: 512c + 2j (z2a) and 512c + 2j + 1
                # (z2b), j<256. Contraction runs over original columns c_g,
                # folded: zp[r2] = y1[r2] + y1[4095-r2], zm = difference;
                # fz[:, k, 0:256] = zp, fz[:, k, 256:512] = zm.
                yv = y[:].rearrange("(a b) n -> a b n", b=2)  # [256, 2, 4096]
                for fhalf in range(2):
                    fz = fza if fhalf == 0 else fzb
                    if fhalf == 0:
                        # odd-half fold tiles; prep interleaves into the even
                        # half's matmul groups below (A2A#1 lands meanwhile)
                        fzb = foldp.tile([128, KT, CB], bf, tag="fB", name="fzb")
                        prep = hook(z2b, fzb, stage1)
                    else:
                        prep = None
                    g = 0
                    for rm in range(2):
                        for nch2 in range(NCH):
                            # quarter-row buffer: filled by 2 psum groups,
                            # stored while the next quarter computes
                            yt = ytp.tile([128, N // 4], bf, tag=f"yt{rm}",
                                          name="yt")
                            ytv = yt[:].rearrange("p (a b) -> p a b", b=2)
                            for blk2, woff in ((0, 0), (1, 256)):
                                wsl = wes if blk2 == 0 else wos
                                psum = psp.tile([128, 512], dt.float32,
                                                tag="ps", name="ps2")
                                for k in range(KT):
                                    nc.tensor.matmul(
                                        psum[:],
                                        fz[:, k,
                                           woff + rm * 128:woff + (rm + 1) * 128],
                                        wsl[nch2][:, k],
                                        start=(k == 0), stop=(k == KT - 1))
                                # true col = 2*(nch2*512 + j) + blk2
                                nc.vector.tensor_copy(ytv[:, :, blk2], psum[:])
                                if prep is not None:
                                    prep(g)
                                g += 1
                            # true local row = 2*(rm*128 + p) + fhalf
                            nc.gpsimd.dma_start(
                                out=yv[rm * 128:(rm + 1) * 128, fhalf,
                                       nch2 * 1024:(nch2 + 1) * 1024],
                                in_=yt[:])

    nc.compile()
    return nc


def _weights():
    n = np.arange(M, dtype=np.float64)
    k = np.arange(M, dtype=np.float64)
    C = np.cos(np.pi * (2.0 * n[None, :] + 1.0) * k[:, None] / (2.0 * M))
    We = np.ascontiguousarray(C[0::2, :KH].T)  # [r', i'']
    Wo = np.ascontiguousarray(C[1::2, :KH].T)
    return We, Wo


def tile3(a):
    return np.ascontiguousarray(rearrange(a, "(m p) n -> p m n", p=128))


def _wslab(Wt):
    # [128, KT, KH] -> [128, NCH, KT, 512] so each resident slab loads as one
    # contiguous 16KB-per-partition DMA.
    t = tile3(Wt)  # [128, KT, KH]
    return np.ascontiguousarray(
        t.reshape(128, KT, NCH, 512).transpose(0, 2, 1, 3))


def _host_inputs():
    bf = ml_dtypes.bfloat16
    We, Wo = _weights()
    return {
        "we": _wslab(We).astype(bf),
        "wo": _wslab(Wo).astype(bf),
        "jrev": np.ascontiguousarray(np.eye(128)[::-1]).astype(bf),
    }


def kernel(x, expkM=None, expkN=None, trace=False):
    bf = ml_dtypes.bfloat16
    x = np.asarray(x, dtype=np.float32).astype(bf)
    if "nc" not in _BUILT:
        _BUILT["nc"] = build_nc()
        _BUILT.update(_host_inputs())
    nc = _BUILT["nc"]
    xrev = x[::-1, :]
    in_maps = []
    for c in range(NC):
        sl = slice(c * CB, (c + 1) * CB)
        xfr = np.stack((tile3(x[:KH, sl]), tile3(xrev[:KH, sl])), axis=2)
        in_maps.append({
            "xfr": np.ascontiguousarray(xfr),
            "we": _BUILT["we"],
            "wo": _BUILT["wo"],
            "jrev": _BUILT["jrev"],
        })
    res = bass_utils.run_bass_kernel_spmd(nc, in_maps, core_ids=list(range(NC)),
                                          trace=trace)
    _BUILT["last_res"] = res
    out = np.concatenate([res.results[c]["y"] for c in range(NC)], axis=0)
    return out.astype(np.float32)


# revision 25
# speedup vs baseline: 1.8153x; 1.1205x over previous
# 2D DCT-II [4096,4096] fp32 on 8 NeuronCores — v5 "two-level fold".
#
# v4 (bf16, SBUF-resident weights, split/overlapped AllToAll) left the PE
# 92% busy, so v5 cuts PE work itself: the even/odd DCT fold is applied
# RECURSIVELY to the even branch, splitting each pass into
#   odd   : U[2e+1]  = Wo  [2048x2048] @ (x[r] - x[~r])           (16 k-tiles)
#   even-a: U[4e2]   = We2a[1024x1024] @ (xp[r2] + xp[~r2])        (8 k-tiles)
#   even-b: U[4e2+2] = We2b[1024x1024] @ (xp[r2] - xp[~r2])        (8 k-tiles)
# for 6M MACs per output column instead of 8M (384 matmuls/pass vs 512) and
# 12.6MB of resident weights instead of 16.8MB. The partition-reversed
# mirrors (x[~r]) come from tiny J-matmuls against the anti-identity.
#
# Orientation: data tiles are lhsT (stationary), cos-weights are rhs
# (moving); pass-1 emits U^T tiles [c-part, row-free]; the AllToAll is split
# into even/odd halves (z1a/z1b), each hidden under the next compute block;
# pass-2 consumes the A2A output directly, with all fold prep interleaved
# into the preceding matmul block so the PE never idles at boundaries.
# Core c owns true rows 512c..512c+511: z1a[c] carries them as
# j<128 -> row 512c+4j (branch a), j>=128 -> 512c+4(j-128)+2 (branch b);
# z1b[c]: j -> 512c+2j+1. Queue placement: input/weight streaming on SP in
# first-use order, staging loads + tr2 + drain y stores on ACT, z1 stores +
# collectives + overlapped y stores on Pool/SWDGE (separate semaphore pool),
# folds/evac on DVE; a 12-deep ev pool absorbs z1-store latency behind the
# weight stream.
import numpy as np
import ml_dtypes
from einops import rearrange
import concourse.bacc as bacc
import concourse.tile as tile
import concourse.mybir as mybir
from concourse import bass_utils

M = N = 4096
NC = 8
CB = 512          # columns per core (pass 1) / rows per core (pass 2)
KH = M // 2       # 2048 level-1 folded contraction length
KT = KH // 128    # 16 K-tiles (odd branch)
KT2 = KT // 2     # 8 K-tiles (level-2 even branches)
NCH = KH // 512   # 4 odd-branch N-chunks of 512

_BUILT = {}


def build_nc(repeat=1, local_sim=False):
    dt = mybir.dt
    bf = dt.bfloat16
    nc = bacc.Bacc("TRN2", target_bir_lowering=False, debug=False, num_devices=NC)

    # packed pass-1 input, mirror-pair bundles: chunk c4 carries, for its
    # two pairs q = 2*c4 + j2, the four planes (Xf[q], Xr[q], Xf[15-q],
    # Xr[15-q]) at e = 4*j2 + (0..3) — so each chunk feeds the complete
    # level-1 AND level-2 fold chain for its k-pairs with no cross-chunk wait.
    xfr = nc.dram_tensor("xfr", [128, 4, 8, CB], bf, kind="ExternalInput")
    wo = nc.dram_tensor("wo", [128, NCH, KT, 512], bf, kind="ExternalInput")
    we2a = nc.dram_tensor("we2a", [128, KT2, 1024], bf, kind="ExternalInput")
    we2b = nc.dram_tensor("we2b", [128, KT2, 1024], bf, kind="ExternalInput")
    jrev = nc.dram_tensor("jrev", [128, 128], bf, kind="ExternalInput")
    y = nc.dram_tensor("y", [CB, N], bf, kind="ExternalOutput")

    with tile.TileContext(nc) as tc:
        with (
            tc.tile_pool(name="dram", bufs=1, space="DRAM") as dram,
            tc.tile_pool(name="wpool", bufs=1) as wpool,
            tc.tile_pool(name="foldp", bufs=1) as foldp,
            tc.tile_pool(name="xst", bufs=2) as xst,
            tc.tile_pool(name="zst", bufs=4) as zst,
            tc.tile_pool(name="evp", bufs=12) as evp,
            tc.tile_pool(name="jp", bufs=1) as jp,
            tc.tile_pool(name="ytp", bufs=1) as ytp,
            tc.tile_pool(name="psp", bufs=4, space="PSUM") as psp,
            tc.tile_pool(name="psj", bufs=3, space="PSUM") as psj,
        ):
            z1a = dram.tile([NC, CB, 256], bf)
            z1b = dram.tile([NC, CB, 256], bf)
            z2a = dram.tile([NC, CB, 256], bf)
            z2b = dram.tile([NC, CB, 256], bf)

            for _rep in range(repeat):
                jt = jp.tile([128, 128], bf, tag="jt")
                nc.scalar.dma_start(out=jt[:], in_=jrev[:])
                # ---- streaming on SP in strict first-use order
                w2a = wpool.tile([128, KT2, 1024], bf, tag="w2a", name="w2a")
                w2b = wpool.tile([128, KT2, 1024], bf, tag="w2b", name="w2b")
                wos = [wpool.tile([128, KT, 512], bf, tag=f"wo{i}", name="wos")
                       for i in range(NCH)]
                xcs = []
                for c4 in range(4):
                    xc = xst.tile([128, 8, CB], bf, tag="xc", name="xc")
                    nc.sync.dma_start(out=xc[:], in_=xfr[:, c4])
                    xcs.append(xc)
                nc.sync.dma_start(out=w2a[:], in_=we2a[:])
                nc.sync.dma_start(out=w2b[:], in_=we2b[:])
                for i in range(NCH):
                    nc.sync.dma_start(out=wos[i][:], in_=wo[:, i])
                # ---- folds, fully pipelined per mirror-pair chunk:
                # level-1: xp = X[r]+X[4095-r], xm = X[r]-X[4095-r];
                # level-2 on the even branch (r2 mirror via J-matmul):
                # xp2[:, k2, 0:512] = xp[r2]+xp[2047-r2], [512:1024] = minus.
                xp = foldp.tile([128, KT, CB], bf, tag="fA", name="xp")
                xm = foldp.tile([128, KT, CB], bf, tag="fB", name="xm")
                xp2 = foldp.tile([128, KT2, 1024], bf, tag="f2", name="xp2")
                for c4 in range(4):
                    xc = xcs[c4]
                    for j2 in range(2):
                        q = 2 * c4 + j2
                        b = 4 * j2
                        nc.vector.tensor_add(xp[:, q], xc[:, b], xc[:, b + 1])
                        nc.vector.tensor_add(xp[:, KT - 1 - q],
                                             xc[:, b + 2], xc[:, b + 3])
                        nc.vector.tensor_sub(xm[:, q], xc[:, b], xc[:, b + 1])
                        nc.vector.tensor_sub(xm[:, KT - 1 - q],
                                             xc[:, b + 2], xc[:, b + 3])
                        prr = psj.tile([128, 512], dt.float32, tag="pj",
                                       name="prr")
                        nc.tensor.matmul(prr[:], jt[:], xp[:, KT - 1 - q],
                                         start=True, stop=True)
                        xq = zst.tile([128, 512], bf, tag="xq", name="xq",
                                      bufs=3)
                        nc.scalar.copy(xq[:], prr[:])
                        nc.vector.tensor_add(xp2[:, q, 0:512], xp[:, q], xq[:])
                        nc.vector.tensor_sub(xp2[:, q, 512:1024],
                                             xp[:, q], xq[:])

                # ---- pass-2 prep emitters (interleaved into the preceding
                # matmul block). Level-1: paired staging loads + J-reversal +
                # fold into fz; level-2: J-reversal of fz's zp half into fz2.
                def emit_load(z2x, stage, kt):
                    zr = z2x[:].rearrange("s (ch p) j -> p (s ch) j", p=128)
                    tf2 = zst.tile([128, 2, 256], bf, tag="tf2", name="tf2",
                                   bufs=8)
                    ta2 = zst.tile([128, 2, 256], bf, tag="ta2", name="ta2",
                                   bufs=8)
                    nc.scalar.dma_start(out=tf2[:], in_=zr[:, kt:kt + 2])
                    nc.scalar.dma_start(out=ta2[:], in_=zr[:, 30 - kt:32 - kt])
                    stage[kt] = (tf2, ta2)

                def emit_jfold(fz, stage, kt):
                    tf2, ta2 = stage[kt]
                    prr = psj.tile([128, 512], dt.float32, tag="pj", name="prr")
                    nc.tensor.matmul(prr[:], jt[:],
                                     ta2[:].rearrange("p t j -> p (t j)"),
                                     start=True, stop=True)
                    tr2 = zst.tile([128, 2, 256], bf, tag="tr2", name="tr2")
                    nc.scalar.copy(tr2[:].rearrange("p t j -> p (t j)"), prr[:])
                    for t in range(2):
                        nc.vector.tensor_add(fz[:, kt + t, 0:256],
                                             tf2[:, t], tr2[:, 1 - t])
                        nc.vector.tensor_sub(fz[:, kt + t, 256:512],
                                             tf2[:, t], tr2[:, 1 - t])

                def emit_jfold2(fz, fz2, k2):
                    prr = psj.tile([128, 256], dt.float32, tag="pj", name="prr")
                    nc.tensor.matmul(prr[:], jt[:], fz[:, KT - 1 - k2, 0:256],
                                     start=True, stop=True)
                    qz = zst.tile([128, 256], bf, tag="qz", name="qz", bufs=4)
                    nc.scalar.copy(qz[:], prr[:])
                    nc.vector.tensor_add(fz2[:, k2, 0:256],
                                         fz[:, k2, 0:256], qz[:])
                    nc.vector.tensor_sub(fz2[:, k2, 256:512],
                                         fz[:, k2, 0:256], qz[:])

                def hook(z2x, fz, fz2, stage, jg):
                    def run(g):
                        if 4 <= g < 8:
                            emit_load(z2x, stage, 4 * (g - 4))
                            emit_load(z2x, stage, 4 * (g - 4) + 2)
                        if jg <= g < jg + 4:
                            emit_jfold(fz, stage, 4 * (g - jg))
                            emit_jfold(fz, stage, 4 * (g - jg) + 2)
                        if jg + 4 <= g < jg + 6:
                            for k2 in range(4 * (g - jg - 4), 4 * (g - jg - 3)):
                                emit_jfold2(fz, fz2, k2)
                    return run

                # ================= pass 1, even branches (-> z1a) =========
                # branch a (+fold, We2a) then b (-fold, We2b); psum[c, e2]
                # splits 4 ways: dest core 4*nch2a+piece, j = br*128 + e2%128
                for br, wt2 in ((0, w2a), (1, w2b)):
                    for nch2a in range(2):
                        for cm in range(CB // 128):
                            psum = psp.tile([128, 512], dt.float32, tag="ps",
                                            name="ps1e")
                            for k2 in range(KT2):
                                nc.tensor.matmul(
                                    psum[:],
                                    xp2[:, k2, br * 512 + cm * 128:
                                        br * 512 + (cm + 1) * 128],
                                    wt2[:, k2, nch2a * 512:(nch2a + 1) * 512],
                                    start=(k2 == 0), stop=(k2 == KT2 - 1))
                            ev = evp.tile([128, 512], bf, tag="ev", name="ev")
                            nc.vector.tensor_copy(ev[:], psum[:])
                            for piece in range(4):
                                nc.gpsimd.dma_start(
                                    out=z1a[4 * nch2a + piece,
                                            cm * 128:(cm + 1) * 128,
                                            br * 128:(br + 1) * 128],
                                    in_=ev[:, piece * 128:(piece + 1) * 128])
                if local_sim:
                    nc.gpsimd.dma_start(out=z2a[:], in_=z1a[:])
                else:
                    nc.gpsimd.collective_compute(
                        "AllToAll", mybir.AluOpType.bypass,
                        replica_groups=[list(range(NC))],
                        ins=[z1a[:].opt()], outs=[z2a[:].opt()])

                # ================= pass 1, odd branch (-> z1b) ============
                fza = foldp.tile([128, KT, CB], bf, tag="fA", name="fza")
                fz2a = foldp.tile([128, KT2, 1024], bf, tag="f2", name="fz2a")
                stage0, stage1 = {}, {}
                prep = hook(z2a, fza, fz2a, stage0, 9)
                g = 0
                for nch in range(NCH):
                    for cm in range(CB // 128):
                        psum = psp.tile([128, 512], dt.float32, tag="ps",
                                        name="ps1o")
                        for k in range(KT):
                            nc.tensor.matmul(psum[:],
                                             xm[:, k, cm * 128:(cm + 1) * 128],
                                             wos[nch][:, k],
                                             start=(k == 0), stop=(k == KT - 1))
                        ev = evp.tile([128, 512], bf, tag="ev", name="ev")
                        nc.vector.tensor_copy(ev[:], psum[:])
                        for piece in range(2):
                            nc.gpsimd.dma_start(
                                out=z1b[2 * nch + piece,
                                        cm * 128:(cm + 1) * 128, :],
                                in_=ev[:, piece * 256:(piece + 1) * 256])
                        prep(g)
                        g += 1
                if local_sim:
                    nc.gpsimd.dma_start(out=z2b[:], in_=z1b[:])
                else:
                    nc.gpsimd.collective_compute(
                        "AllToAll", mybir.AluOpType.bypass,
                        replica_groups=[list(range(NC))],
                        ins=[z1b[:].opt()], outs=[z2b[:].opt()])

                # ================= pass 2 =================
                # per fhalf: 256 owned rows (batch dim j); contraction over
                # original columns c_g, folded once (odd) or twice (even).
                yv4 = y[:].rearrange("(a b) n -> a b n", b=4)  # [128, 4, 4096]
                yv2 = y[:].rearrange("(a b) n -> a b n", b=2)  # [256, 2, 4096]
                fzb = fz2b = None
                for fhalf in range(2):
                    if fhalf == 0:
                        fz, fz2 = fza, fz2a
                        fzb = foldp.tile([128, KT, CB], bf, tag="fB", name="fzb")
                        fz2b = foldp.tile([128, KT2, 1024], bf, tag="f2",
                                          name="fz2b")
                        prep = hook(z2b, fzb, fz2b, stage1, 10)
                    else:
                        fz, fz2 = fzb, fz2b
                        prep = None
                    g = 0
                    for rm in range(2):
                        for h in range(2):
                            # half-row buffer: true cols [2048h, 2048h+2048)
                            yt = ytp.tile([128, N // 2], bf, tag=f"yt{rm}",
                                          name="yt")
                            yt2 = yt[:].rearrange("p (a b) -> p a b", b=2)
                            yt4 = yt[:].rearrange("p (a b) -> p a b", b=4)
                            specs = [
                                ("o", 2 * h), ("o", 2 * h + 1),
                                ("a", h), ("b", h),
                            ]
                            for kind, idx in specs:
                                psum = psp.tile([128, 512], dt.float32,
                                                tag="ps", name="ps2")
                                if kind == "o":
                                    for k in range(KT):
                                        nc.tensor.matmul(
                                            psum[:],
                                            fz[:, k, 256 + rm * 128:
                                               256 + (rm + 1) * 128],
                                            wos[idx][:, k],
                                            start=(k == 0), stop=(k == KT - 1))
                                    # true col = 2e+1, e = idx*512 + q
                                    d = idx - 2 * h
                                    nc.vector.tensor_copy(
                                        yt2[:, d * 512:(d + 1) * 512, 1],
                                        psum[:])
                                else:
                                    woff = 0 if kind == "a" else 256
                                    wt2 = w2a if kind == "a" else w2b
                                    for k2 in range(KT2):
                                        nc.tensor.matmul(
                                            psum[:],
                                            fz2[:, k2, woff + rm * 128:
                                                woff + (rm + 1) * 128],
                                            wt2[:, k2,
                                                idx * 512:(idx + 1) * 512],
                                            start=(k2 == 0),
                                            stop=(k2 == KT2 - 1))
                                    # true col = 4e2 (+2 for branch b)
                                    nc.vector.tensor_copy(
                                        yt4[:, :, 0 if kind == "a" else 2],
                                        psum[:])
                                if prep is not None:
                                    prep(g)
                                g += 1
                            if fhalf == 0:
                                # rm0 -> rows 4j, rm1 -> rows 4j+2 (SWDGE:
                                # off the HWDGE rotation, can't throttle the
                                # f1 staging loads)
                                nc.gpsimd.dma_start(
                                    out=yv4[:, 2 * rm,
                                            h * 2048:(h + 1) * 2048],
                                    in_=yt[:])
                            else:
                                # rows 2(rm*128+p)+1; fast HWDGE for drain
                                nc.scalar.dma_start(
                                    out=yv2[rm * 128:(rm + 1) * 128, 1,
                                            h * 2048:(h + 1) * 2048],
                                    in_=yt[:])

    nc.compile()
    return nc


def _weights():
    r = np.arange(KH, dtype=np.float64)
    e = np.arange(KH, dtype=np.float64)
    Wo = np.cos(np.pi * (2.0 * r[:, None] + 1.0) * (2.0 * e[None, :] + 1.0)
                / (2.0 * M))                        # [r, e] odd branch
    r2 = np.arange(1024, dtype=np.float64)
    e2 = np.arange(1024, dtype=np.float64)
    We2a = np.cos(np.pi * (2.0 * r2[:, None] + 1.0) * e2[None, :] / 2048.0)
    We2b = np.cos(np.pi * (2.0 * r2[:, None] + 1.0) * (2.0 * e2[None, :] + 1.0)
                  / 4096.0)
    return Wo, We2a, We2b


def tile3(a):
    return np.ascontiguousarray(rearrange(a, "(m p) n -> p m n", p=128))


def _host_inputs():
    bf = ml_dtypes.bfloat16
    Wo, We2a, We2b = _weights()
    wo4 = tile3(Wo).reshape(128, KT, NCH, 512).transpose(0, 2, 1, 3)
    return {
        "wo": np.ascontiguousarray(wo4).astype(bf),
        "we2a": tile3(We2a).astype(bf),
        "we2b": tile3(We2b).astype(bf),
        "jrev": np.ascontiguousarray(np.eye(128)[::-1]).astype(bf),
    }


def kernel(x, expkM=None, expkN=None, trace=False):
    bf = ml_dtypes.bfloat16
    x = np.asarray(x, dtype=np.float32).astype(bf)
    if "nc" not in _BUILT:
        _BUILT["nc"] = build_nc()
        _BUILT.update(_host_inputs())
    nc = _BUILT["nc"]
    xrev = x[::-1, :]
    in_maps = []
    for c in range(NC):
        sl = slice(c * CB, (c + 1) * CB)
        xf_t = tile3(x[:KH, sl])      # [128, KT, CB]
        xr_t = tile3(xrev[:KH, sl])
        xfr = np.empty((128, 4, 8, CB), dtype=xf_t.dtype)
        for c4 in range(4):
            for j2 in range(2):
                q = 2 * c4 + j2
                xfr[:, c4, 4 * j2 + 0] = xf_t[:, q]
                xfr[:, c4, 4 * j2 + 1] = xr_t[:, q]
                xfr[:, c4, 4 * j2 + 2] = xf_t[:, KT - 1 - q]
                xfr[:, c4, 4 * j2 + 3] = xr_t[:, KT - 1 - q]
        in_maps.append({
            "xfr": np.ascontiguousarray(xfr),
            "wo": _BUILT["wo"],
            "we2a": _BUILT["we2a"],
            "we2b": _BUILT["we2b"],
            "jrev": _BUILT["jrev"],
        })
    res = bass_utils.run_bass_kernel_spmd(nc, in_maps, core_ids=list(range(NC)),
                                          trace=trace)
    _BUILT["last_res"] = res
    out = np.concatenate([res.results[c]["y"] for c in range(NC)], axis=0)
    return out.astype(np.float32)


# revision 31
# speedup vs baseline: 1.8833x; 1.0375x over previous
# 2D DCT-II [4096,4096] fp32 on 8 NeuronCores — v5 "two-level fold".
#
# v4 (bf16, SBUF-resident weights, split/overlapped AllToAll) left the PE
# 92% busy, so v5 cuts PE work itself: the even/odd DCT fold is applied
# RECURSIVELY to the even branch, splitting each pass into
#   odd   : U[2e+1]  = Wo  [2048x2048] @ (x[r] - x[~r])           (16 k-tiles)
#   even-a: U[4e2]   = We2a[1024x1024] @ (xp[r2] + xp[~r2])        (8 k-tiles)
#   even-b: U[4e2+2] = We2b[1024x1024] @ (xp[r2] - xp[~r2])        (8 k-tiles)
# for 6M MACs per output column instead of 8M (384 matmuls/pass vs 512) and
# 12.6MB of resident weights instead of 16.8MB. The partition-reversed
# mirrors (x[~r]) come from tiny J-matmuls against the anti-identity.
#
# Orientation: data tiles are lhsT (stationary), cos-weights are rhs
# (moving); pass-1 emits U^T tiles [c-part, row-free]; the AllToAll is split
# into even/odd halves (z1a/z1b), each hidden under the next compute block;
# pass-2 consumes the A2A output directly, with all fold prep interleaved
# into the preceding matmul block so the PE never idles at boundaries.
# Core c owns true rows 512c..512c+511: z1a[c] carries them as
# j<128 -> row 512c+4j (branch a), j>=128 -> 512c+4(j-128)+2 (branch b);
# z1b[c]: j -> 512c+2j+1. Queue placement: input/weight streaming on SP in
# first-use order, staging loads + tr2 + drain y stores on ACT, z1 stores +
# collectives + overlapped y stores on Pool/SWDGE (separate semaphore pool),
# folds/evac on DVE; a 12-deep ev pool absorbs z1-store latency behind the
# weight stream.
import numpy as np
import ml_dtypes
from einops import rearrange
import concourse.bacc as bacc
import concourse.tile as tile
import concourse.mybir as mybir
from concourse import bass_utils

M = N = 4096
NC = 8
CB = 512          # columns per core (pass 1) / rows per core (pass 2)
KH = M // 2       # 2048 level-1 folded contraction length
KT = KH // 128    # 16 K-tiles (odd branch)
KT2 = KT // 2     # 8 K-tiles (level-2 even branches)
NCH = KH // 512   # 4 odd-branch N-chunks of 512

_BUILT = {}


def build_nc(repeat=1, local_sim=False):
    dt = mybir.dt
    bf = dt.bfloat16
    nc = bacc.Bacc("TRN2", target_bir_lowering=False, debug=False, num_devices=NC)

    # packed pass-1 input, mirror-pair bundles: chunk c4 carries, for its
    # two pairs q = 2*c4 + j2, the four planes (Xf[q], Xr[q], Xf[15-q],
    # Xr[15-q]) at e = 4*j2 + (0..3) — so each chunk feeds the complete
    # level-1 AND level-2 fold chain for its k-pairs with no cross-chunk wait.
    xfr = nc.dram_tensor("xfr", [128, 4, 8, CB], bf, kind="ExternalInput")
    wo = nc.dram_tensor("wo", [128, NCH, KT, 512], bf, kind="ExternalInput")
    we2a = nc.dram_tensor("we2a", [128, KT2, 1024], bf, kind="ExternalInput")
    we2b = nc.dram_tensor("we2b", [128, KT2, 1024], bf, kind="ExternalInput")
    jrev = nc.dram_tensor("jrev", [128, 128], bf, kind="ExternalInput")
    y = nc.dram_tensor("y", [CB, N], bf, kind="ExternalOutput")

    with tile.TileContext(nc) as tc:
        with (
            tc.tile_pool(name="dram", bufs=1, space="DRAM") as dram,
            tc.tile_pool(name="wpool", bufs=1) as wpool,
            tc.tile_pool(name="foldp", bufs=1) as foldp,
            tc.tile_pool(name="xst", bufs=2) as xst,
            tc.tile_pool(name="zst", bufs=4) as zst,
            tc.tile_pool(name="evp", bufs=12) as evp,
            tc.tile_pool(name="jp", bufs=1) as jp,
            tc.tile_pool(name="ytp", bufs=1) as ytp,
            tc.tile_pool(name="psp", bufs=4, space="PSUM") as psp,
            tc.tile_pool(name="psj", bufs=3, space="PSUM") as psj,
        ):
            z1a = dram.tile([NC, CB, 256], bf)
            z1b = dram.tile([NC, CB, 256], bf)
            z2a = dram.tile([NC, CB, 256], bf)
            z2b = dram.tile([NC, CB, 256], bf)

            for _rep in range(repeat):
                jt = jp.tile([128, 128], bf, tag="jt")
                nc.scalar.dma_start(out=jt[:], in_=jrev[:])
                # ---- streaming on SP in strict first-use order
                w2a = wpool.tile([128, KT2, 1024], bf, tag="w2a", name="w2a")
                w2b = wpool.tile([128, KT2, 1024], bf, tag="w2b", name="w2b")
                wos = [wpool.tile([128, KT, 512], bf, tag=f"wo{i}", name="wos")
                       for i in range(NCH)]
                xcs = []
                for c4 in range(4):
                    xc = xst.tile([128, 8, CB], bf, tag="xc", name="xc")
                    nc.sync.dma_start(out=xc[:], in_=xfr[:, c4])
                    xcs.append(xc)
                nc.sync.dma_start(out=w2a[:], in_=we2a[:])
                nc.sync.dma_start(out=w2b[:], in_=we2b[:])
                # odd-branch slabs in quarter-chunks: small transfers clear
                # the ring-fairness scrum quickly, so each group's k-slices
                # arrive in demand order instead of one scrambled big slab
                for i in range(NCH):
                    for c in range(0, KT, 4):
                        nc.sync.dma_start(out=wos[i][:, c:c + 4],
                                          in_=wo[:, i, c:c + 4])
                # ---- folds, fully pipelined per mirror-pair chunk:
                # level-1: xp = X[r]+X[4095-r], xm = X[r]-X[4095-r];
                # level-2 on the even branch (r2 mirror via J-matmul):
                # xp2[:, k2, 0:512] = xp[r2]+xp[2047-r2], [512:1024] = minus.
                xp = foldp.tile([128, KT, CB], bf, tag="fA", name="xp")
                xm = foldp.tile([128, KT, CB], bf, tag="fB", name="xm")
                xp2 = foldp.tile([128, KT2, 1024], bf, tag="f2", name="xp2")
                for c4 in range(4):
                    xc = xcs[c4]
                    for j2 in range(2):
                        q = 2 * c4 + j2
                        b = 4 * j2
                        nc.vector.tensor_add(xp[:, q], xc[:, b], xc[:, b + 1])
                        nc.vector.tensor_add(xp[:, KT - 1 - q],
                                             xc[:, b + 2], xc[:, b + 3])
                        nc.vector.tensor_sub(xm[:, q], xc[:, b], xc[:, b + 1])
                        nc.vector.tensor_sub(xm[:, KT - 1 - q],
                                             xc[:, b + 2], xc[:, b + 3])
                        prr = psj.tile([128, 512], dt.float32, tag="pj",
                                       name="prr")
                        nc.tensor.matmul(prr[:], jt[:], xp[:, KT - 1 - q],
                                         start=True, stop=True)
                        xq = zst.tile([128, 512], bf, tag="xq", name="xq",
                                      bufs=3)
                        nc.scalar.copy(xq[:], prr[:])
                        nc.vector.tensor_add(xp2[:, q, 0:512], xp[:, q], xq[:])
                        nc.vector.tensor_sub(xp2[:, q, 512:1024],
                                             xp[:, q], xq[:])

                # ---- pass-2 prep emitters (interleaved into the preceding
                # matmul block). Level-1: paired staging loads + J-reversal +
                # fold into fz; level-2: J-reversal of fz's zp half into fz2.
                def emit_load(z2x, stage, kt):
                    zr = z2x[:].rearrange("s (ch p) j -> p (s ch) j", p=128)
                    tf2 = zst.tile([128, 2, 256], bf, tag="tf2", name="tf2",
                                   bufs=8)
                    ta2 = zst.tile([128, 2, 256], bf, tag="ta2", name="ta2",
                                   bufs=8)
                    nc.scalar.dma_start(out=tf2[:], in_=zr[:, kt:kt + 2])
                    nc.scalar.dma_start(out=ta2[:], in_=zr[:, 30 - kt:32 - kt])
                    stage[kt] = (tf2, ta2)

                def emit_jfold(fz, stage, kt):
                    tf2, ta2 = stage[kt]
                    prr = psj.tile([128, 512], dt.float32, tag="pj", name="prr")
                    nc.tensor.matmul(prr[:], jt[:],
                                     ta2[:].rearrange("p t j -> p (t j)"),
                                     start=True, stop=True)
                    tr2 = zst.tile([128, 2, 256], bf, tag="tr2", name="tr2")
                    nc.scalar.copy(tr2[:].rearrange("p t j -> p (t j)"), prr[:])
                    for t in range(2):
                        nc.vector.tensor_add(fz[:, kt + t, 0:256],
                                             tf2[:, t], tr2[:, 1 - t])
                        nc.vector.tensor_sub(fz[:, kt + t, 256:512],
                                             tf2[:, t], tr2[:, 1 - t])

                def emit_jfold2(fz, fz2, k2):
                    prr = psj.tile([128, 256], dt.float32, tag="pj", name="prr")
                    nc.tensor.matmul(prr[:], jt[:], fz[:, KT - 1 - k2, 0:256],
                                     start=True, stop=True)
                    qz = zst.tile([128, 256], bf, tag="qz", name="qz", bufs=4)
                    nc.scalar.copy(qz[:], prr[:])
                    nc.vector.tensor_add(fz2[:, k2, 0:256],
                                         fz[:, k2, 0:256], qz[:])
                    nc.vector.tensor_sub(fz2[:, k2, 256:512],
                                         fz[:, k2, 0:256], qz[:])

                def hook(z2x, fz, fz2, stage, jg):
                    def run(g):
                        if 4 <= g < 8:
                            emit_load(z2x, stage, 4 * (g - 4))
                            emit_load(z2x, stage, 4 * (g - 4) + 2)
                        if jg <= g < jg + 4:
                            emit_jfold(fz, stage, 4 * (g - jg))
                            emit_jfold(fz, stage, 4 * (g - jg) + 2)
                        if jg + 4 <= g < jg + 6:
                            for k2 in range(4 * (g - jg - 4), 4 * (g - jg - 3)):
                                emit_jfold2(fz, fz2, k2)
                    return run

                # ================= pass 1, even branches (-> z1a) =========
                # branch a (+fold, We2a) then b (-fold, We2b); psum[c, e2]
                # splits 4 ways: dest core 4*nch2a+piece, j = br*128 + e2%128
                for br, wt2 in ((0, w2a), (1, w2b)):
                    for nch2a in range(2):
                        for cm in range(CB // 128):
                            psum = psp.tile([128, 512], dt.float32, tag="ps",
                                            name="ps1e")
                            for k2 in range(KT2):
                                nc.tensor.matmul(
                                    psum[:],
                                    xp2[:, k2, br * 512 + cm * 128:
                                        br * 512 + (cm + 1) * 128],
                                    wt2[:, k2, nch2a * 512:(nch2a + 1) * 512],
                                    start=(k2 == 0), stop=(k2 == KT2 - 1))
                            ev = evp.tile([128, 512], bf, tag="ev", name="ev")
                            nc.vector.tensor_copy(ev[:], psum[:])
                            for piece in range(4):
                                eng = nc.sync if piece % 2 == 0 else nc.scalar
                                eng.dma_start(
                                    out=z1a[4 * nch2a + piece,
                                            cm * 128:(cm + 1) * 128,
                                            br * 128:(br + 1) * 128],
                                    in_=ev[:, piece * 128:(piece + 1) * 128])
                if local_sim:
                    nc.gpsimd.dma_start(out=z2a[:], in_=z1a[:])
                else:
                    nc.gpsimd.collective_compute(
                        "AllToAll", mybir.AluOpType.bypass,
                        replica_groups=[list(range(NC))],
                        ins=[z1a[:].opt()], outs=[z2a[:].opt()])

                # ================= pass 1, odd branch (-> z1b) ============
                fza = foldp.tile([128, KT, CB], bf, tag="fA", name="fza")
                fz2a = foldp.tile([128, KT2, 1024], bf, tag="f2", name="fz2a")
                stage0, stage1 = {}, {}
                prep = hook(z2a, fza, fz2a, stage0, 9)
                g = 0
                for nch in range(NCH):
                    for cm in range(CB // 128):
                        psum = psp.tile([128, 512], dt.float32, tag="ps",
                                        name="ps1o")
                        for k in range(KT):
                            nc.tensor.matmul(psum[:],
                                             xm[:, k, cm * 128:(cm + 1) * 128],
                                             wos[nch][:, k],
                                             start=(k == 0), stop=(k == KT - 1))
                        ev = evp.tile([128, 512], bf, tag="ev", name="ev")
                        nc.vector.tensor_copy(ev[:], psum[:])
                        for piece in range(2):
                            nc.gpsimd.dma_start(
                                out=z1b[2 * nch + piece,
                                        cm * 128:(cm + 1) * 128, :],
                                in_=ev[:, piece * 256:(piece + 1) * 256])
                        prep(g)
                        g += 1
                if local_sim:
                    nc.gpsimd.dma_start(out=z2b[:], in_=z1b[:])
                else:
                    nc.gpsimd.collective_compute(
                        "AllToAll", mybir.AluOpType.bypass,
                        replica_groups=[list(range(NC))],
                        ins=[z1b[:].opt()], outs=[z2b[:].opt()])

                # ================= pass 2 =================
                # per fhalf: 256 owned rows (batch dim j); contraction over
                # original columns c_g, folded once (odd) or twice (even).
                yv4 = y[:].rearrange("(a b) n -> a b n", b=4)  # [128, 4, 4096]
                yv2 = y[:].rearrange("(a b) n -> a b n", b=2)  # [256, 2, 4096]
                fzb = fz2b = None
                for fhalf in range(2):
                    if fhalf == 0:
                        fz, fz2 = fza, fz2a
                        fzb = foldp.tile([128, KT, CB], bf, tag="fB", name="fzb")
                        fz2b = foldp.tile([128, KT2, 1024], bf, tag="f2",
                                          name="fz2b")
                        prep = hook(z2b, fzb, fz2b, stage1, 10)
                    else:
                        fz, fz2 = fzb, fz2b
                        prep = None
                    g = 0
                    for rm in range(2):
                        for h in range(2):
                            # half-row buffer: true cols [2048h, 2048h+2048)
                            yt = ytp.tile([128, N // 2], bf, tag=f"yt{rm}",
                                          name="yt")
                            yt2 = yt[:].rearrange("p (a b) -> p a b", b=2)
                            yt4 = yt[:].rearrange("p (a b) -> p a b", b=4)
                            specs = [
                                ("o", 2 * h), ("o", 2 * h + 1),
                                ("a", h), ("b", h),
                            ]
                            for kind, idx in specs:
                                psum = psp.tile([128, 512], dt.float32,
                                                tag="ps", name="ps2")
                                if kind == "o":
                                    for k in range(KT):
                                        nc.tensor.matmul(
                                            psum[:],
                                            fz[:, k, 256 + rm * 128:
                                               256 + (rm + 1) * 128],
                                            wos[idx][:, k],
                                            start=(k == 0), stop=(k == KT - 1))
                                    # true col = 2e+1, e = idx*512 + q
                                    d = idx - 2 * h
                                    nc.vector.tensor_copy(
                                        yt2[:, d * 512:(d + 1) * 512, 1],
                                        psum[:])
                                else:
                                    woff = 0 if kind == "a" else 256
                                    wt2 = w2a if kind == "a" else w2b
                                    for k2 in range(KT2):
                                        nc.tensor.matmul(
                                            psum[:],
                                            fz2[:, k2, woff + rm * 128:
                                                woff + (rm + 1) * 128],
                                            wt2[:, k2,
                                                idx * 512:(idx + 1) * 512],
                                            start=(k2 == 0),
                                            stop=(k2 == KT2 - 1))
                                    # true col = 4e2 (+2 for branch b)
                                    nc.vector.tensor_copy(
                                        yt4[:, :, 0 if kind == "a" else 2],
                                        psum[:])
                                if prep is not None:
                                    prep(g)
                                g += 1
                            if fhalf == 0:
                                # rm0 -> rows 4j, rm1 -> rows 4j+2 (SWDGE:
                                # off the HWDGE rotation, can't throttle the
                                # f1 staging loads)
                                nc.gpsimd.dma_start(
                                    out=yv4[:, 2 * rm,
                                            h * 2048:(h + 1) * 2048],
                                    in_=yt[:])
                            else:
                                # rows 2(rm*128+p)+1; fast HWDGE for drain
                                nc.scalar.dma_start(
                                    out=yv2[rm * 128:(rm + 1) * 128, 1,
                                            h * 2048:(h + 1) * 2048],
                                    in_=yt[:])

    nc.compile()
    return nc


def _weights():
    r = np.arange(KH, dtype=np.float64)
    e = np.arange(KH, dtype=np.float64)
    Wo = np.cos(np.pi * (2.0 * r[:, None] + 1.0) * (2.0 * e[None, :] + 1.0)
                / (2.0 * M))                        # [r, e] odd branch
    r2 = np.arange(1024, dtype=np.float64)
    e2 = np.arange(1024, dtype=np.float64)
    We2a = np.cos(np.pi * (2.0 * r2[:, None] + 1.0) * e2[None, :] / 2048.0)
    We2b = np.cos(np.pi * (2.0 * r2[:, None] + 1.0) * (2.0 * e2[None, :] + 1.0)
                  / 4096.0)
    return Wo, We2a, We2b


def tile3(a):
    return np.ascontiguousarray(rearrange(a, "(m p) n -> p m n", p=128))


def _host_inputs():
    bf = ml_dtypes.bfloat16
    Wo, We2a, We2b = _weights()
    wo4 = tile3(Wo).reshape(128, KT, NCH, 512).transpose(0, 2, 1, 3)
    return {
        "wo": np.ascontiguousarray(wo4).astype(bf),
        "we2a": tile3(We2a).astype(bf),
        "we2b": tile3(We2b).astype(bf),
        "jrev": np.ascontiguousarray(np.eye(128)[::-1]).astype(bf),
    }


def kernel(x, expkM=None, expkN=None, trace=False):
    bf = ml_dtypes.bfloat16
    x = np.asarray(x, dtype=np.float32).astype(bf)
    if "nc" not in _BUILT:
        _BUILT["nc"] = build_nc()
        _BUILT.update(_host_inputs())
    nc = _BUILT["nc"]
    xrev = x[::-1, :]
    in_maps = []
    for c in range(NC):
        sl = slice(c * CB, (c + 1) * CB)
        xf_t = tile3(x[:KH, sl])      # [128, KT, CB]
        xr_t = tile3(xrev[:KH, sl])
        xfr = np.empty((128, 4, 8, CB), dtype=xf_t.dtype)
        for c4 in range(4):
            for j2 in range(2):
                q = 2 * c4 + j2
                xfr[:, c4, 4 * j2 + 0] = xf_t[:, q]
                xfr[:, c4, 4 * j2 + 1] = xr_t[:, q]
                xfr[:, c4, 4 * j2 + 2] = xf_t[:, KT - 1 - q]
                xfr[:, c4, 4 * j2 + 3] = xr_t[:, KT - 1 - q]
        in_maps.append({
            "xfr": np.ascontiguousarray(xfr),
            "wo": _BUILT["wo"],
            "we2a": _BUILT["we2a"],
            "we2b": _BUILT["we2b"],
            "jrev": _BUILT["jrev"],
        })
    res = bass_utils.run_bass_kernel_spmd(nc, in_maps, core_ids=list(range(NC)),
                                          trace=trace)
    _BUILT["last_res"] = res
    out = np.concatenate([res.results[c]["y"] for c in range(NC)], axis=0)
    return out.astype(np.float32)


# revision 34
# speedup vs baseline: 1.9912x; 1.0573x over previous
# 2D DCT-II [4096,4096] fp32 on 8 NeuronCores — v5 "two-level fold".
#
# v4 (bf16, SBUF-resident weights, split/overlapped AllToAll) left the PE
# 92% busy, so v5 cuts PE work itself: the even/odd DCT fold is applied
# RECURSIVELY to the even branch, splitting each pass into
#   odd   : U[2e+1]  = Wo  [2048x2048] @ (x[r] - x[~r])           (16 k-tiles)
#   even-a: U[4e2]   = We2a[1024x1024] @ (xp[r2] + xp[~r2])        (8 k-tiles)
#   even-b: U[4e2+2] = We2b[1024x1024] @ (xp[r2] - xp[~r2])        (8 k-tiles)
# for 6M MACs per output column instead of 8M (384 matmuls/pass vs 512) and
# 12.6MB of resident weights instead of 16.8MB. The partition-reversed
# mirrors (x[~r]) come from tiny J-matmuls against the anti-identity.
#
# Orientation: data tiles are lhsT (stationary), cos-weights are rhs
# (moving); pass-1 emits U^T tiles [c-part, row-free]; the AllToAll is split
# into even/odd halves (z1a/z1b), each hidden under the next compute block;
# pass-2 consumes the A2A output directly, with all fold prep interleaved
# into the preceding matmul block so the PE never idles at boundaries.
# Core c owns true rows 512c..512c+511: z1a[c] carries them as
# j<128 -> row 512c+4j (branch a), j>=128 -> 512c+4(j-128)+2 (branch b);
# z1b[c]: j -> 512c+2j+1. Queue placement: input/weight streaming on SP in
# first-use order, staging loads + tr2 + drain y stores on ACT, z1 stores +
# collectives + overlapped y stores on Pool/SWDGE (separate semaphore pool),
# folds/evac on DVE; a 12-deep ev pool absorbs z1-store latency behind the
# weight stream.
import numpy as np
import ml_dtypes
from einops import rearrange
import concourse.bacc as bacc
import concourse.tile as tile
import concourse.mybir as mybir
from concourse import bass_utils

M = N = 4096
NC = 8
CB = 512          # columns per core (pass 1) / rows per core (pass 2)
KH = M // 2       # 2048 level-1 folded contraction length
KT = KH // 128    # 16 K-tiles (odd branch)
KT2 = KT // 2     # 8 K-tiles (level-2 even branches)
NCH = KH // 512   # 4 odd-branch N-chunks of 512

_BUILT = {}


def build_nc(repeat=1, local_sim=False):
    dt = mybir.dt
    bf = dt.bfloat16
    nc = bacc.Bacc("TRN2", target_bir_lowering=False, debug=False, num_devices=NC)

    # packed pass-1 input, mirror-pair bundles: chunk c4 carries, for its
    # two pairs q = 2*c4 + j2, the four planes (Xf[q], Xr[q], Xf[15-q],
    # Xr[15-q]) at e = 4*j2 + (0..3) — so each chunk feeds the complete
    # level-1 AND level-2 fold chain for its k-pairs with no cross-chunk wait.
    xfr = nc.dram_tensor("xfr", [128, 4, 8, CB], bf, kind="ExternalInput")
    wo = nc.dram_tensor("wo", [128, NCH, KT, 512], bf, kind="ExternalInput")
    we2a = nc.dram_tensor("we2a", [128, KT2, 1024], bf, kind="ExternalInput")
    we2b = nc.dram_tensor("we2b", [128, KT2, 1024], bf, kind="ExternalInput")
    jrev = nc.dram_tensor("jrev", [128, 128], bf, kind="ExternalInput")
    y = nc.dram_tensor("y", [CB, N], bf, kind="ExternalOutput")

    with tile.TileContext(nc) as tc:
        with (
            tc.tile_pool(name="dram", bufs=1, space="DRAM") as dram,
            tc.tile_pool(name="wpool", bufs=1) as wpool,
            tc.tile_pool(name="foldp", bufs=1) as foldp,
            tc.tile_pool(name="xst", bufs=2) as xst,
            tc.tile_pool(name="zst", bufs=4) as zst,
            tc.tile_pool(name="evp", bufs=12) as evp,
            tc.tile_pool(name="jp", bufs=1) as jp,
            tc.tile_pool(name="ytp", bufs=1) as ytp,
            tc.tile_pool(name="psp", bufs=4, space="PSUM") as psp,
            tc.tile_pool(name="psj", bufs=3, space="PSUM") as psj,
        ):
            z1a = dram.tile([NC, CB, 256], bf)
            z1b = dram.tile([NC, CB, 256], bf)
            z2a = dram.tile([NC, CB, 256], bf)
            z2b = dram.tile([NC, CB, 256], bf)

            for _rep in range(repeat):
                jt = jp.tile([128, 128], bf, tag="jt")
                nc.scalar.dma_start(out=jt[:], in_=jrev[:])
                # ---- streaming on SP in strict first-use order
                w2a = wpool.tile([128, KT2, 1024], bf, tag="w2a", name="w2a")
                w2b = wpool.tile([128, KT2, 1024], bf, tag="w2b", name="w2b")
                wos = [wpool.tile([128, KT, 512], bf, tag=f"wo{i}", name="wos")
                       for i in range(NCH)]
                xcs = []
                for c4 in range(4):
                    xc = xst.tile([128, 8, CB], bf, tag="xc", name="xc")
                    nc.sync.dma_start(out=xc[:], in_=xfr[:, c4])
                    xcs.append(xc)
                # even-a's first groups need only w2a's first e2-half;
                # splitting lets them start ~3us earlier
                nc.sync.dma_start(out=w2a[:, :, 0:512], in_=we2a[:, :, 0:512])
                nc.sync.dma_start(out=w2a[:, :, 512:1024],
                                  in_=we2a[:, :, 512:1024])
                nc.sync.dma_start(out=w2b[:], in_=we2b[:])
                nc.sync.dma_start(out=wos[0][:], in_=wo[:, 0])
                nc.sync.dma_start(out=wos[1][:], in_=wo[:, 1])
                # ---- folds, fully pipelined per mirror-pair chunk:
                # level-1: xp = X[r]+X[4095-r], xm = X[r]-X[4095-r];
                # level-2 on the even branch (r2 mirror via J-matmul):
                # xp2[:, k2, 0:512] = xp[r2]+xp[2047-r2], [512:1024] = minus.
                xp = foldp.tile([128, KT, CB], bf, tag="fA", name="xp")
                xm = foldp.tile([128, KT, CB], bf, tag="fB", name="xm")
                xp2 = foldp.tile([128, KT2, 1024], bf, tag="f2", name="xp2")
                for c4 in range(4):
                    xc = xcs[c4]
                    for j2 in range(2):
                        q = 2 * c4 + j2
                        b = 4 * j2
                        nc.vector.tensor_add(xp[:, q], xc[:, b], xc[:, b + 1])
                        nc.vector.tensor_add(xp[:, KT - 1 - q],
                                             xc[:, b + 2], xc[:, b + 3])
                        nc.vector.tensor_sub(xm[:, q], xc[:, b], xc[:, b + 1])
                        nc.vector.tensor_sub(xm[:, KT - 1 - q],
                                             xc[:, b + 2], xc[:, b + 3])
                        prr = psj.tile([128, 512], dt.float32, tag="pj",
                                       name="prr")
                        nc.tensor.matmul(prr[:], jt[:], xp[:, KT - 1 - q],
                                         start=True, stop=True)
                        xq = zst.tile([128, 512], bf, tag="xq", name="xq",
                                      bufs=3)
                        nc.scalar.copy(xq[:], prr[:])
                        nc.vector.tensor_add(xp2[:, q, 0:512], xp[:, q], xq[:])
                        nc.vector.tensor_sub(xp2[:, q, 512:1024],
                                             xp[:, q], xq[:])
                # wo2/wo3 aren't needed until deep into the odd block; riding
                # ACT behind the J2 xq copies delays their dispatch ~20us so
                # they don't starve the front (x, w2a/b, wo0/1, z1a stores)
                nc.scalar.dma_start(out=wos[2][:], in_=wo[:, 2])
                nc.scalar.dma_start(out=wos[3][:], in_=wo[:, 3])

                # ---- pass-2 prep emitters (interleaved into the preceding
                # matmul block). Level-1: paired staging loads + J-reversal +
                # fold into fz; level-2: J-reversal of fz's zp half into fz2.
                def emit_load(z2x, stage, kt):
                    zr = z2x[:].rearrange("s (ch p) j -> p (s ch) j", p=128)
                    tf2 = zst.tile([128, 2, 256], bf, tag="tf2", name="tf2",
                                   bufs=8)
                    ta2 = zst.tile([128, 2, 256], bf, tag="ta2", name="ta2",
                                   bufs=8)
                    nc.scalar.dma_start(out=tf2[:], in_=zr[:, kt:kt + 2])
                    nc.scalar.dma_start(out=ta2[:], in_=zr[:, 30 - kt:32 - kt])
                    stage[kt] = (tf2, ta2)

                def emit_jfold(fz, stage, kt):
                    tf2, ta2 = stage[kt]
                    prr = psj.tile([128, 512], dt.float32, tag="pj", name="prr")
                    nc.tensor.matmul(prr[:], jt[:],
                                     ta2[:].rearrange("p t j -> p (t j)"),
                                     start=True, stop=True)
                    tr2 = zst.tile([128, 2, 256], bf, tag="tr2", name="tr2")
                    nc.scalar.copy(tr2[:].rearrange("p t j -> p (t j)"), prr[:])
                    for t in range(2):
                        nc.vector.tensor_add(fz[:, kt + t, 0:256],
                                             tf2[:, t], tr2[:, 1 - t])
                        nc.vector.tensor_sub(fz[:, kt + t, 256:512],
                                             tf2[:, t], tr2[:, 1 - t])

                def emit_jfold2(fz, fz2, k2):
                    prr = psj.tile([128, 256], dt.float32, tag="pj", name="prr")
                    nc.tensor.matmul(prr[:], jt[:], fz[:, KT - 1 - k2, 0:256],
                                     start=True, stop=True)
                    qz = zst.tile([128, 256], bf, tag="qz", name="qz", bufs=4)
                    nc.scalar.copy(qz[:], prr[:])
                    nc.vector.tensor_add(fz2[:, k2, 0:256],
                                         fz[:, k2, 0:256], qz[:])
                    nc.vector.tensor_sub(fz2[:, k2, 256:512],
                                         fz[:, k2, 0:256], qz[:])

                def hook(z2x, fz, fz2, stage, jg):
                    def run(g):
                        if 4 <= g < 8:
                            emit_load(z2x, stage, 4 * (g - 4))
                            emit_load(z2x, stage, 4 * (g - 4) + 2)
                        if jg <= g < jg + 4:
                            emit_jfold(fz, stage, 4 * (g - jg))
                            emit_jfold(fz, stage, 4 * (g - jg) + 2)
                        if jg + 4 <= g < jg + 6:
                            for k2 in range(4 * (g - jg - 4), 4 * (g - jg - 3)):
                                emit_jfold2(fz, fz2, k2)
                    return run

                # ================= pass 1, even branches (-> z1a) =========
                # branch a (+fold, We2a) then b (-fold, We2b); psum[c, e2]
                # splits 4 ways: dest core 4*nch2a+piece, j = br*128 + e2%128
                for br, wt2 in ((0, w2a), (1, w2b)):
                    for nch2a in range(2):
                        for cm in range(CB // 128):
                            psum = psp.tile([128, 512], dt.float32, tag="ps",
                                            name="ps1e")
                            for k2 in range(KT2):
                                nc.tensor.matmul(
                                    psum[:],
                                    xp2[:, k2, br * 512 + cm * 128:
                                        br * 512 + (cm + 1) * 128],
                                    wt2[:, k2, nch2a * 512:(nch2a + 1) * 512],
                                    start=(k2 == 0), stop=(k2 == KT2 - 1))
                            ev = evp.tile([128, 512], bf, tag="ev", name="ev")
                            nc.vector.tensor_copy(ev[:], psum[:])
                            for piece in range(4):
                                eng = nc.sync if piece % 2 == 0 else nc.scalar
                                eng.dma_start(
                                    out=z1a[4 * nch2a + piece,
                                            cm * 128:(cm + 1) * 128,
                                            br * 128:(br + 1) * 128],
                                    in_=ev[:, piece * 128:(piece + 1) * 128])
                if local_sim:
                    nc.gpsimd.dma_start(out=z2a[:], in_=z1a[:])
                else:
                    nc.gpsimd.collective_compute(
                        "AllToAll", mybir.AluOpType.bypass,
                        replica_groups=[list(range(NC))],
                        ins=[z1a[:].opt()], outs=[z2a[:].opt()])

                # ================= pass 1, odd branch (-> z1b) ============
                fza = foldp.tile([128, KT, CB], bf, tag="fA", name="fza")
                fz2a = foldp.tile([128, KT2, 1024], bf, tag="f2", name="fz2a")
                stage0, stage1 = {}, {}
                prep = hook(z2a, fza, fz2a, stage0, 9)
                g = 0
                for nch in range(NCH):
                    for cm in range(CB // 128):
                        psum = psp.tile([128, 512], dt.float32, tag="ps",
                                        name="ps1o")
                        for k in range(KT):
                            nc.tensor.matmul(psum[:],
                                             xm[:, k, cm * 128:(cm + 1) * 128],
                                             wos[nch][:, k],
                                             start=(k == 0), stop=(k == KT - 1))
                        ev = evp.tile([128, 512], bf, tag="ev", name="ev")
                        nc.vector.tensor_copy(ev[:], psum[:])
                        for piece in range(2):
                            nc.gpsimd.dma_start(
                                out=z1b[2 * nch + piece,
                                        cm * 128:(cm + 1) * 128, :],
                                in_=ev[:, piece * 256:(piece + 1) * 256])
                        prep(g)
                        g += 1
                if local_sim:
                    nc.gpsimd.dma_start(out=z2b[:], in_=z1b[:])
                else:
                    nc.gpsimd.collective_compute(
                        "AllToAll", mybir.AluOpType.bypass,
                        replica_groups=[list(range(NC))],
                        ins=[z1b[:].opt()], outs=[z2b[:].opt()])

                # ================= pass 2 =================
                # per fhalf: 256 owned rows (batch dim j); contraction over
                # original columns c_g, folded once (odd) or twice (even).
                yv4 = y[:].rearrange("(a b) n -> a b n", b=4)  # [128, 4, 4096]
                yv2 = y[:].rearrange("(a b) n -> a b n", b=2)  # [256, 2, 4096]
                fzb = fz2b = None
                for fhalf in range(2):
                    if fhalf == 0:
                        fz, fz2 = fza, fz2a
                        fzb = foldp.tile([128, KT, CB], bf, tag="fB", name="fzb")
                        fz2b = foldp.tile([128, KT2, 1024], bf, tag="f2",
                                          name="fz2b")
                        prep = hook(z2b, fzb, fz2b, stage1, 10)
                    else:
                        fz, fz2 = fzb, fz2b
                        prep = None
                    g = 0
                    for rm in range(2):
                        for h in range(2):
                            # half-row buffer: true cols [2048h, 2048h+2048)
                            yt = ytp.tile([128, N // 2], bf, tag=f"yt{rm}",
                                          name="yt")
                            yt2 = yt[:].rearrange("p (a b) -> p a b", b=2)
                            yt4 = yt[:].rearrange("p (a b) -> p a b", b=4)
                            specs = [
                                ("o", 2 * h), ("o", 2 * h + 1),
                                ("a", h), ("b", h),
                            ]
                            for kind, idx in specs:
                                psum = psp.tile([128, 512], dt.float32,
                                                tag="ps", name="ps2")
                                if kind == "o":
                                    for k in range(KT):
                                        nc.tensor.matmul(
                                            psum[:],
                                            fz[:, k, 256 + rm * 128:
                                               256 + (rm + 1) * 128],
                                            wos[idx][:, k],
                                            start=(k == 0), stop=(k == KT - 1))
                                    # true col = 2e+1, e = idx*512 + q
                                    d = idx - 2 * h
                                    nc.vector.tensor_copy(
                                        yt2[:, d * 512:(d + 1) * 512, 1],
                                        psum[:])
                                else:
                                    woff = 0 if kind == "a" else 256
                                    wt2 = w2a if kind == "a" else w2b
                                    for k2 in range(KT2):
                                        nc.tensor.matmul(
                                            psum[:],
                                            fz2[:, k2, woff + rm * 128:
                                                woff + (rm + 1) * 128],
                                            wt2[:, k2,
                                                idx * 512:(idx + 1) * 512],
                                            start=(k2 == 0),
                                            stop=(k2 == KT2 - 1))
                                    # true col = 4e2 (+2 for branch b)
                                    nc.vector.tensor_copy(
                                        yt4[:, :, 0 if kind == "a" else 2],
                                        psum[:])
                                if prep is not None:
                                    prep(g)
                                g += 1
                            if fhalf == 0:
                                # rm0 -> rows 4j, rm1 -> rows 4j+2 (SWDGE:
                                # off the HWDGE rotation, can't throttle the
                                # f1 staging loads)
                                nc.gpsimd.dma_start(
                                    out=yv4[:, 2 * rm,
                                            h * 2048:(h + 1) * 2048],
                                    in_=yt[:])
                            else:
                                # rows 2(rm*128+p)+1; fast HWDGE for drain
                                nc.scalar.dma_start(
                                    out=yv2[rm * 128:(rm + 1) * 128, 1,
                                            h * 2048:(h + 1) * 2048],
                                    in_=yt[:])

    nc.compile()
    return nc


def _weights():
    r = np.arange(KH, dtype=np.float64)
    e = np.arange(KH, dtype=np.float64)
    Wo = np.cos(np.pi * (2.0 * r[:, None] + 1.0) * (2.0 * e[None, :] + 1.0)
                / (2.0 * M))                        # [r, e] odd branch
    r2 = np.arange(1024, dtype=np.float64)
    e2 = np.arange(1024, dtype=np.float64)
    We2a = np.cos(np.pi * (2.0 * r2[:, None] + 1.0) * e2[None, :] / 2048.0)
    We2b = np.cos(np.pi * (2.0 * r2[:, None] + 1.0) * (2.0 * e2[None, :] + 1.0)
                  / 4096.0)
    return Wo, We2a, We2b


def tile3(a):
    return np.ascontiguousarray(rearrange(a, "(m p) n -> p m n", p=128))


def _host_inputs():
    bf = ml_dtypes.bfloat16
    Wo, We2a, We2b = _weights()
    wo4 = tile3(Wo).reshape(128, KT, NCH, 512).transpose(0, 2, 1, 3)
    return {
        "wo": np.ascontiguousarray(wo4).astype(bf),
        "we2a": tile3(We2a).astype(bf),
        "we2b": tile3(We2b).astype(bf),
        "jrev": np.ascontiguousarray(np.eye(128)[::-1]).astype(bf),
    }


def kernel(x, expkM=None, expkN=None, trace=False):
    bf = ml_dtypes.bfloat16
    x = np.asarray(x, dtype=np.float32).astype(bf)
    if "nc" not in _BUILT:
        _BUILT["nc"] = build_nc()
        _BUILT.update(_host_inputs())
    nc = _BUILT["nc"]
    xrev = x[::-1, :]
    in_maps = []
    for c in range(NC):
        sl = slice(c * CB, (c + 1) * CB)
        xf_t = tile3(x[:KH, sl])      # [128, KT, CB]
        xr_t = tile3(xrev[:KH, sl])
        xfr = np.empty((128, 4, 8, CB), dtype=xf_t.dtype)
        for c4 in range(4):
            for j2 in range(2):
                q = 2 * c4 + j2
                xfr[:, c4, 4 * j2 + 0] = xf_t[:, q]
                xfr[:, c4, 4 * j2 + 1] = xr_t[:, q]
                xfr[:, c4, 4 * j2 + 2] = xf_t[:, KT - 1 - q]
                xfr[:, c4, 4 * j2 + 3] = xr_t[:, KT - 1 - q]
        in_maps.append({
            "xfr": np.ascontiguousarray(xfr),
            "wo": _BUILT["wo"],
            "we2a": _BUILT["we2a"],
            "we2b": _BUILT["we2b"],
            "jrev": _BUILT["jrev"],
        })
    res = bass_utils.run_bass_kernel_spmd(nc, in_maps, core_ids=list(range(NC)),
                                          trace=trace)
    _BUILT["last_res"] = res
    out = np.concatenate([res.results[c]["y"] for c in range(NC)], axis=0)
    return out.astype(np.float32)
